# revision 10
# baseline (speedup 1.0000x reference)
"""DISCO S2 convolution (nn_DISCOBlock_57801669869705) on 8 Trainium2 NeuronCores.

out[b,o,to,q] = sum_{c,k} w[o,c,k] * sum_{w,p} psi[k,to,w,p] * x[b,c,ti[to,w],(p+q)%P]

Mapping: for each output latitude row `to` and each active longitude-shift tap
(latitude-pair j, dp), a TensorE matmul accumulates into PSUM:
    out[:, (q,b)] += WPsi[(m,c), o].T @ xg[(m,c), (q+dp, b)]
with contraction over 128 partitions = (pair member m, channel c).
WPsi[(m,c), o] = sum_k psi[k,to,w(j,m),dp] * weight[o,c,k] is a host-side
transform of the small weight tensor; xg holds the latitudinally gathered,
longitudinally haloed input rows (host-side layout of x), both in bf16.

Tap pairing: adjacent taps (j,dp) and (j,dp+1) share xg rows, so they are
fused into ONE matmul with M=128 = (o, which-tap): PSUM rows 0:64 hold tap
dp's output, rows 64:128 hold tap dp+1's output over an N=362 window; a
final DVE add with a 2-column (one longitude) shift merges the halves.
This nearly halves TensorE cycles (720 paired + 55 single matmuls/core vs
1495). bf16 operands halve the streamed weight/input bytes; PSUM stays f32.

Sharding: the 91 output rows are grouped into 12 "slots" of <=8 rows; the rows
of a slot are computed simultaneously by the 8 cores (one row per core) under a
shared per-slot tap template (union of the rows' taps; absent taps get zero
coefficients). Grouping and the per-slot pairing of the 9 latitude-window rows
into 128-partition contraction blocks are jointly optimized by a DP to
minimize total matmul count.
"""

import math
from functools import lru_cache

import numpy as np

B, C, O = 2, 64, 64
NLAT, P = 91, 180
NR, NPHI = 5, 6
K = (NR - 1) * NPHI + 1
NCORE = 8
NSLOT = 12
NJ = 5  # pair slots per latitude window (4 pairs + 1 single)
NPAIR = 362  # moving-dim width of a paired matmul: B*(P+1)
WP_CHUNK = 8192  # wp cols per streamed weight-block DMA (16KB/partition bf16)
WP_FIRST = 1024  # small first chunk to shorten the preamble


def _compute_psi():
    theta_cut = 4.0 * math.pi / (NLAT - 1)
    half = int(math.ceil(theta_cut / (math.pi / (NLAT - 1))))
    theta = np.pi * np.arange(NLAT) / (NLAT - 1)
    phi_in = 2.0 * np.pi * np.arange(P) / P
    offs = np.arange(-half, half + 1)
    ti_raw = np.arange(NLAT)[:, None] + offs[None, :]
    valid = (ti_raw >= 0) & (ti_raw < NLAT)
    ti_idx = np.clip(ti_raw, 0, NLAT - 1)
    to = theta[:, None, None]
    ti = theta[ti_idx][:, :, None]
    ph = phi_in[None, None, :]
    xx = np.cos(to) * np.sin(ti) * np.cos(ph) - np.sin(to) * np.cos(ti)
    yy = np.sin(ti) * np.sin(ph)
    zz = np.sin(to) * np.sin(ti) * np.cos(ph) + np.cos(to) * np.cos(ti)
    r = np.arccos(np.clip(zz, -1.0, 1.0))
    az = np.mod(np.arctan2(yy, xx), 2.0 * np.pi)
    dr = theta_cut / (NR - 1)
    dphi = 2.0 * np.pi / NPHI
    inside = (r <= theta_cut) & valid[:, :, None]
    psi = np.zeros((K,) + r.shape)
    psi[0] = np.where(inside, np.maximum(0.0, 1.0 - r / dr), 0.0)
    for ir in range(1, NR):
        rad = np.maximum(0.0, 1.0 - np.abs(r - ir * dr) / dr)
        for ip in range(NPHI):
            d = np.abs(np.mod(az - ip * dphi + np.pi, 2.0 * np.pi) - np.pi)
            ang = np.maximum(0.0, 1.0 - d / dphi)
            psi[1 + (ir - 1) * NPHI + ip] = np.where(inside, rad * ang, 0.0)
    quad = np.sin(theta) * (np.pi / (NLAT - 1)) * (2.0 * np.pi / P)
    psi = psi * quad[ti_idx][None, :, :, None]
    return psi.astype(np.float32), ti_idx.astype(np.int32), 2 * half + 1


def _best_matching(u):
    """u: [W, P] bool. Return (cost, groups) — 4 pairs + 1 single over w=0..8
    minimizing sum over groups of |union of member activity|."""
    Wn = u.shape[0]
    M = np.zeros((Wn, Wn), dtype=np.int64)
    for a in range(Wn):
        for b in range(a + 1, Wn):
            M[a, b] = int((u[a] | u[b]).sum())
    s = np.array([int(u[w].sum()) for w in range(Wn)])
    INF = 10**12

    @lru_cache(maxsize=None)
    def f(mask, single_used):
        if mask == 0:
            return 0, ()
        a = (mask & -mask).bit_length() - 1
        rest = mask & ~(1 << a)
        best = (INF, ())
        for b in range(a + 1, Wn):
            if rest >> b & 1:
                c, pl = f(rest & ~(1 << b), single_used)
                if M[a, b] + c < best[0]:
                    best = (M[a, b] + c, pl + ((a, b),))
        if not single_used:
            c, pl = f(rest, True)
            if s[a] + c < best[0]:
                best = (s[a] + c, pl + ((a, None),))
        return best

    c, pl = f((1 << Wn) - 1, False)
    f.cache_clear()
    return c, list(pl)


def _build_plan():
    psi, ti_idx, W = _compute_psi()
    dpval = np.where(np.arange(P) < P // 2, np.arange(P), np.arange(P) - P)
    active = (psi != 0).any(axis=0)  # [To, W, P]

    cnt = active.reshape(NLAT, -1).sum(axis=1)
    order = list(np.argsort(-cnt, kind="stable"))
    n = len(order)
    INF = 10**12
    cost = np.full((n + 1, n + 1), INF, dtype=np.int64)
    pairings = {}
    for i in range(n):
        u = np.zeros((W, P), dtype=bool)
        for j in range(i + 1, min(i + 9, n + 1)):
            u = u | active[order[j - 1]]
            c, pl = _best_matching(u)
            cost[i][j] = c
            pairings[(i, j)] = pl
    dp = np.full((n + 1, NSLOT + 1), INF, dtype=np.int64)
    par = np.zeros((n + 1, NSLOT + 1), dtype=np.int64)
    dp[0][0] = 0
    for j in range(1, NSLOT + 1):
        for i in range(1, n + 1):
            for i0 in range(max(0, i - 8), i):
                v = dp[i0][j - 1] + cost[i0][i]
                if v < dp[i][j]:
                    dp[i][j] = v
                    par[i][j] = i0
    bounds = []
    i = n
    for j in range(NSLOT, 0, -1):
        i0 = par[i][j]
        bounds.append((i0, i))
        i = i0
    bounds = bounds[::-1]

    row_of = -np.ones((NCORE, NSLOT), dtype=np.int64)
    slot_pairs, templates, halos = [], [], []
    for s, (i0, i1) in enumerate(bounds):
        rows = [order[t] for t in range(i0, i1)]
        for ci, t in enumerate(rows):
            row_of[ci, s] = t
        pairs = pairings[(i0, i1)]
        assert len(pairs) == NJ
        slot_pairs.append(pairs)
        u = active[rows].any(axis=0)  # [W, P]
        tap_list = []
        for j, (wa, wb) in enumerate(pairs):
            ws = [w for w in (wa, wb) if w is not None]
            act_j = u[ws].any(axis=0)  # [P]
            pp = np.nonzero(act_j)[0]
            for dp_ in sorted(dpval[pp].tolist()):
                tap_list.append((j, dp_))
        templates.append(tap_list)
        halos.append(max((abs(d) for _, d in tap_list), default=0))

    qpads = [P + 2 * h for h in halos]
    offs = np.cumsum([0] + [NJ * B * qp for qp in qpads]).tolist()

    # --- tap pairing: fuse (j,dp) with (j,dp+1) into one M=128 matmul ---
    # per slot: list of ('P', j, dp) pairs first (any pair initializes the
    # full [128, NPAIR] psum region via start=True), then ('S', j, dp).
    tgmap = {}
    tg = 0
    for s in range(NSLOT):
        for (j, dp_) in templates[s]:
            tgmap[(s, j, dp_)] = tg
            tg += 1
    from collections import defaultdict
    slot_descs = []
    wp_col = 0
    for s in range(NSLOT):
        byj = defaultdict(list)
        for j, dp_ in templates[s]:
            byj[j].append(dp_)
        prs, sgl = [], []
        for j in sorted(byj):
            dps = sorted(byj[j])
            i = 0
            while i < len(dps):
                if i + 1 < len(dps) and dps[i + 1] == dps[i] + 1:
                    prs.append((j, dps[i]))
                    i += 2
                else:
                    sgl.append((j, dps[i]))
                    i += 1
        assert prs, f"slot {s} has no paired tap"
        descs = []
        for j, dp_ in prs:
            descs.append(("P", j, dp_, wp_col))
            wp_col += 2 * O
        for j, dp_ in sgl:
            descs.append(("S", j, dp_, wp_col))
            wp_col += O
        slot_descs.append(descs)

    # wp chunk boundaries (in cols), aligned to matmul blocks
    flat = [d for ds in slot_descs for d in ds]
    chunk_bounds = [0]
    target = WP_FIRST
    for kind, _, _, col in flat:
        ncols = 2 * O if kind == "P" else O
        if col + ncols - chunk_bounds[-1] > target:
            chunk_bounds.append(col)
            target = WP_CHUNK
    chunk_bounds.append(wp_col)

    return dict(psi=psi, ti_idx=ti_idx, W=W, row_of=row_of, templates=templates,
                slot_pairs=slot_pairs, halos=halos, qpads=qpads, offs=offs,
                xg_cols=int(offs[-1]), tgmap=tgmap, slot_descs=slot_descs,
                wp_cols=int(wp_col), chunk_bounds=chunk_bounds,
                t_total=int(sum(len(t) for t in templates)))


_PLAN = None
_NC = None


def _get_plan():
    global _PLAN
    if _PLAN is None:
        _PLAN = _build_plan()
    return _PLAN


def _build_nc(plan):
    import concourse.bacc as bacc
    import concourse.mybir as mybir
    import concourse.tile as tile

    f32 = mybir.dt.float32
    bf16 = mybir.dt.bfloat16

    halos = plan["halos"]
    qpads = plan["qpads"]
    offs = plan["offs"]
    XG_COLS = plan["xg_cols"]
    WP_COLS = plan["wp_cols"]
    slot_descs = plan["slot_descs"]
    chunk_bounds = plan["chunk_bounds"]

    nc = bacc.Bacc("TRN2", target_bir_lowering=False, debug=False,
                   num_devices=NCORE)
    xg_d = nc.declare_dram_parameter("xg", [128, XG_COLS], bf16, isOutput=False)
    wp_d = nc.declare_dram_parameter("wp", [128, WP_COLS], bf16, isOutput=False)
    # both tap halves are shipped out; the host does the 1-longitude
    # shifted add (cross-partition adds are rejected by the BIR verifier)
    out_d = nc.declare_dram_parameter("out", [128, NSLOT * NPAIR], f32,
                                      isOutput=True)

    with tile.TileContext(nc) as tc:
        with (
            tc.tile_pool(name="xg", bufs=1) as xgp,
            tc.tile_pool(name="wp", bufs=3) as wpp,
            tc.tile_pool(name="ps", bufs=4, space="PSUM") as psp,
            tc.tile_pool(name="outp", bufs=2) as outp,
        ):
            # xg tiles on the gpsimd (SWDGE) queue so inputs load in
            # parallel with the weight chunks (sync/HWDGE). Slot 0 is split
            # per pair-slot j so the very first matmul only waits on a sliver.
            xg_ts = []
            for s in range(NSLOT):
                qp = qpads[s]
                if s == 0:
                    pieces = []
                    for j in range(NJ):
                        pj = xgp.tile([128, B * qp], bf16, tag=f"xg0_{j}")
                        nc.gpsimd.dma_start(
                            pj[:], xg_d[:, offs[s] + j * B * qp:
                                        offs[s] + (j + 1) * B * qp])
                        pieces.append(pj)
                    xg_ts.append(pieces)
                else:
                    seg = xgp.tile([128, NJ * B * qp], bf16, tag=f"xg{s}")
                    nc.gpsimd.dma_start(seg[:], xg_d[:, offs[s]:offs[s + 1]])
                    xg_ts.append(seg)
            out_t = outp.tile([128, NSLOT * NPAIR], f32)

            cidx = 0
            wp_t = None
            for s in range(NSLOT):
                descs = slot_descs[s]
                qp = qpads[s]
                h = halos[s]
                acc = psp.tile([128, NPAIR], f32)
                for i, (kind, j, dp, col) in enumerate(descs):
                    ncols = 2 * O if kind == "P" else O
                    if col == chunk_bounds[cidx]:
                        ccols = chunk_bounds[cidx + 1] - chunk_bounds[cidx]
                        wp_t = wpp.tile([128, WP_CHUNK], bf16, tag="wp")
                        # alternate queues so the weight stream gets the
                        # bandwidth of two DMA rings
                        eng = nc.sync if cidx % 2 == 0 else nc.scalar
                        eng.dma_start(
                            wp_t[:, :ccols],
                            wp_d[:, chunk_bounds[cidx]:chunk_bounds[cidx + 1]])
                        cidx += 1
                    coff = col - chunk_bounds[cidx - 1]
                    lhsT = wp_t[:, coff:coff + ncols]
                    if s == 0:
                        xv = xg_ts[0][j]
                        base = B * (h + dp)
                    else:
                        xv = xg_ts[s]
                        base = j * B * qp + B * (h + dp)
                    if kind == "P":
                        rhs = xv[:, base: base + NPAIR]
                        out_ap = acc[:, :]
                    else:
                        rhs = xv[:, base: base + B * P]
                        out_ap = acc[0:O, 0:B * P]
                    nc.tensor.matmul(out_ap, lhsT, rhs,
                                     start=(i == 0), stop=(i == len(descs) - 1))
                nc.vector.tensor_copy(
                    out_t[:, s * NPAIR:(s + 1) * NPAIR], acc[:, :])
                # spread output DMAs over both weight queues
                oeng = nc.scalar if s % 2 == 0 else nc.sync
                oeng.dma_start(
                    out_d[:, s * NPAIR:(s + 1) * NPAIR],
                    out_t[:, s * NPAIR:(s + 1) * NPAIR])

    nc.compile()
    return nc


def _get_nc():
    global _NC
    if _NC is None:
        _NC = _build_nc(_get_plan())
    return _NC


def _build_core_inputs(plan, x, weight):
    import ml_dtypes

    psi = plan["psi"]
    ti_idx = plan["ti_idx"]
    row_of = plan["row_of"]
    templates = plan["templates"]
    slot_pairs = plan["slot_pairs"]
    halos = plan["halos"]
    qpads = plan["qpads"]
    offs = plan["offs"]
    XG_COLS = plan["xg_cols"]
    T = plan["t_total"]
    tgmap = plan["tgmap"]
    slot_descs = plan["slot_descs"]
    WP_COLS = plan["wp_cols"]

    coef = np.zeros((NCORE, T, 2, K), dtype=np.float32)
    tg = 0
    for s in range(NSLOT):
        pairs = slot_pairs[s]
        for (j, dp) in templates[s]:
            p = dp % P
            members = pairs[j]
            for core in range(NCORE):
                to = row_of[core, s]
                if to < 0:
                    continue
                for m in range(2):
                    w_ = members[m] if m < len(members) else None
                    if w_ is not None:
                        coef[core, tg, m] = psi[:, to, w_, p]
            tg += 1
    wk = np.ascontiguousarray(weight.transpose(2, 1, 0)).reshape(K, C, O)
    # [NCORE, T, 2, C, O] -> per tap a [128=(m,c), 64=o] stationary block
    wp_all = np.einsum("ntmk,kco->ntmco", coef, wk, optimize=True)

    # assemble wp in matmul-stream order (pair blocks: tap dp then dp+1)
    tg_order = []
    for s in range(NSLOT):
        for kind, j, dp, _ in slot_descs[s]:
            tg_order.append(tgmap[(s, j, dp)])
            if kind == "P":
                tg_order.append(tgmap[(s, j, dp + 1)])
    tg_order = np.array(tg_order, dtype=np.int64)
    wps = []
    for n in range(NCORE):
        blocks = wp_all[n][tg_order].reshape(len(tg_order), 128, O)
        wp = np.ascontiguousarray(blocks.transpose(1, 0, 2).reshape(128, -1))
        assert wp.shape[1] == WP_COLS
        wps.append(wp.astype(ml_dtypes.bfloat16))

    xgs = []
    for core in range(NCORE):
        xg = np.zeros((128, XG_COLS), dtype=np.float32)
        for s in range(NSLOT):
            to = row_of[core, s]
            if to < 0:
                continue
            qp = qpads[s]
            h = halos[s]
            qq = (np.arange(qp) - h) % P
            for j, members in enumerate(slot_pairs[s]):
                for m in range(2):
                    w_ = members[m] if m < len(members) else None
                    if w_ is None:
                        continue
                    ti = ti_idx[to, w_]
                    blk = x[:, :, ti, :][:, :, qq]  # [b, c, qp]
                    # column layout (qq, b) so a dp-shifted rhs is contiguous
                    xg[m * 64:(m + 1) * 64,
                       offs[s] + j * B * qp: offs[s] + (j + 1) * B * qp] = (
                        blk.transpose(1, 2, 0).reshape(C, qp * B))
        xgs.append(xg.astype(ml_dtypes.bfloat16))
    return xgs, wps


def kernel(x, weight):
    from concourse.bass_utils import run_bass_kernel_spmd

    x = np.ascontiguousarray(np.asarray(x, dtype=np.float32))
    weight = np.ascontiguousarray(np.asarray(weight, dtype=np.float32))
    plan = _get_plan()
    nc = _get_nc()
    xgs, wps = _build_core_inputs(plan, x, weight)
    in_maps = [{"xg": xgs[i], "wp": wps[i]} for i in range(NCORE)]
    res = run_bass_kernel_spmd(nc, in_maps, list(range(NCORE)))

    out = np.zeros((B, O, NLAT, P), dtype=np.float32)
    row_of = plan["row_of"]
    for core in range(NCORE):
        oc = np.asarray(res.results[core]["out"]).reshape(128, NSLOT, NPAIR)
        merged = oc[0:O, :, 0:B * P] + oc[O:128, :, B:B * P + B]
        mg = merged.reshape(O, NSLOT, P, B)
        for s in range(NSLOT):
            to = row_of[core, s]
            if to >= 0:
                out[:, :, to, :] = mg[:, s, :, :].transpose(2, 0, 1)
    return out


def _numpy_sim(x, weight):
    """Host replica of the device program (for validation)."""
    plan = _get_plan()
    xgs, wps = _build_core_inputs(plan, x, weight)
    slot_descs = plan["slot_descs"]
    halos = plan["halos"]
    qpads = plan["qpads"]
    offs = plan["offs"]
    out = np.zeros((B, O, NLAT, P), dtype=np.float32)
    row_of = plan["row_of"]
    for core in range(NCORE):
        xg = xgs[core].astype(np.float32)
        wp = wps[core].astype(np.float32)
        oc = np.zeros((O, NSLOT, P, B), dtype=np.float32)
        for s in range(NSLOT):
            qp = qpads[s]
            h = halos[s]
            acc = np.zeros((128, NPAIR), dtype=np.float32)
            for (kind, j, dp, col) in slot_descs[s]:
                base = offs[s] + j * B * qp + B * (h + dp)
                if kind == "P":
                    rhs = xg[:, base: base + NPAIR]
                    acc += wp[:, col:col + 2 * O].T @ rhs
                else:
                    rhs = xg[:, base: base + B * P]
                    acc[0:O, 0:B * P] += wp[:, col:col + O].T @ rhs
            merged = acc[0:O, 0:B * P] + acc[O:2 * O, B:B * P + B]
            oc[:, s] = merged.reshape(O, P, B)
        for s in range(NSLOT):
            to = row_of[core, s]
            if to >= 0:
                out[:, :, to, :] = oc[:, s, :, :].transpose(2, 0, 1)
    return out


if __name__ == "__main__":
    plan = _get_plan()
    npair = sum(1 for ds in plan["slot_descs"] for d in ds if d[0] == "P")
    nsgl = sum(1 for ds in plan["slot_descs"] for d in ds if d[0] == "S")
    cyc = npair * NPAIR + nsgl * B * P
    print("t_total:", plan["t_total"], "pairs:", npair, "singles:", nsgl,
          "matmul cycles:", cyc, "->", cyc * 0.41667 / 1e3, "us")
    print("xg MB:", plan["xg_cols"] * 128 * 2 / 1e6,
          "wp MB:", plan["wp_cols"] * 128 * 2 / 1e6)
    d = np.load("/tmp/ref_io.npz")
    got = _numpy_sim(d["x"], d["weight"])
    exp = d["expected"]
    rel = np.linalg.norm((got - exp).ravel()) / np.linalg.norm(exp.ravel())
    print("numpy-sim rel err:", rel)


# revision 12
# speedup vs baseline: 1.1927x; 1.1927x over previous
"""DISCO S2 convolution (nn_DISCOBlock_57801669869705) on 8 Trainium2 NeuronCores.

out[b,o,to,q] = sum_{c,k} w[o,c,k] * sum_{w,p} psi[k,to,w,p] * x[b,c,ti[to,w],(p+q)%P]

Mapping: for each output latitude row `to` and each active longitude-shift tap
(latitude-pair j, dp), a TensorE matmul accumulates into PSUM:
    out[:, (q,b)] += WPsi[(m,c), o].T @ xg[(m,c), (q+dp, b)]
with contraction over 128 partitions = (pair member m, channel c).
WPsi[(m,c), o] = sum_k psi[k,to,w(j,m),dp] * weight[o,c,k] is a host-side
transform of the small weight tensor; xg holds the latitudinally gathered,
longitudinally haloed input rows (host-side layout of x), both in bf16.

Tap pairing: adjacent taps (j,dp) and (j,dp+1) share xg rows, so they are
fused into ONE matmul with M=128 = (o, which-tap): PSUM rows 0:64 hold tap
dp's output, rows 64:128 hold tap dp+1's output over an N=362 window; a
final DVE add with a 2-column (one longitude) shift merges the halves.
This nearly halves TensorE cycles (720 paired + 55 single matmuls/core vs
1495). bf16 operands halve the streamed weight/input bytes; PSUM stays f32.

Sharding: the 91 output rows are grouped into 12 "slots" of <=8 rows; the rows
of a slot are computed simultaneously by the 8 cores (one row per core) under a
shared per-slot tap template (union of the rows' taps; absent taps get zero
coefficients). Grouping and the per-slot pairing of the 9 latitude-window rows
into 128-partition contraction blocks are jointly optimized by a DP to
minimize total matmul count.
"""

import math
from functools import lru_cache

import numpy as np

B, C, O = 2, 64, 64
NLAT, P = 91, 180
NR, NPHI = 5, 6
K = (NR - 1) * NPHI + 1
NCORE = 8
NSLOT = 12
NJ = 5  # pair slots per latitude window (4 pairs + 1 single)
NPAIR = 362  # moving-dim width of a paired matmul: B*(P+1)
WP_CHUNK = 8192  # wp cols per streamed weight-block DMA (16KB/partition bf16)
WP_FIRST = 1024  # small first chunk to shorten the preamble


def _compute_psi():
    theta_cut = 4.0 * math.pi / (NLAT - 1)
    half = int(math.ceil(theta_cut / (math.pi / (NLAT - 1))))
    theta = np.pi * np.arange(NLAT) / (NLAT - 1)
    phi_in = 2.0 * np.pi * np.arange(P) / P
    offs = np.arange(-half, half + 1)
    ti_raw = np.arange(NLAT)[:, None] + offs[None, :]
    valid = (ti_raw >= 0) & (ti_raw < NLAT)
    ti_idx = np.clip(ti_raw, 0, NLAT - 1)
    to = theta[:, None, None]
    ti = theta[ti_idx][:, :, None]
    ph = phi_in[None, None, :]
    xx = np.cos(to) * np.sin(ti) * np.cos(ph) - np.sin(to) * np.cos(ti)
    yy = np.sin(ti) * np.sin(ph)
    zz = np.sin(to) * np.sin(ti) * np.cos(ph) + np.cos(to) * np.cos(ti)
    r = np.arccos(np.clip(zz, -1.0, 1.0))
    az = np.mod(np.arctan2(yy, xx), 2.0 * np.pi)
    dr = theta_cut / (NR - 1)
    dphi = 2.0 * np.pi / NPHI
    inside = (r <= theta_cut) & valid[:, :, None]
    psi = np.zeros((K,) + r.shape)
    psi[0] = np.where(inside, np.maximum(0.0, 1.0 - r / dr), 0.0)
    for ir in range(1, NR):
        rad = np.maximum(0.0, 1.0 - np.abs(r - ir * dr) / dr)
        for ip in range(NPHI):
            d = np.abs(np.mod(az - ip * dphi + np.pi, 2.0 * np.pi) - np.pi)
            ang = np.maximum(0.0, 1.0 - d / dphi)
            psi[1 + (ir - 1) * NPHI + ip] = np.where(inside, rad * ang, 0.0)
    quad = np.sin(theta) * (np.pi / (NLAT - 1)) * (2.0 * np.pi / P)
    psi = psi * quad[ti_idx][None, :, :, None]
    return psi.astype(np.float32), ti_idx.astype(np.int32), 2 * half + 1


def _best_matching(u):
    """u: [W, P] bool. Return (cost, groups) — 4 pairs + 1 single over w=0..8
    minimizing sum over groups of |union of member activity|."""
    Wn = u.shape[0]
    M = np.zeros((Wn, Wn), dtype=np.int64)
    for a in range(Wn):
        for b in range(a + 1, Wn):
            M[a, b] = int((u[a] | u[b]).sum())
    s = np.array([int(u[w].sum()) for w in range(Wn)])
    INF = 10**12

    @lru_cache(maxsize=None)
    def f(mask, single_used):
        if mask == 0:
            return 0, ()
        a = (mask & -mask).bit_length() - 1
        rest = mask & ~(1 << a)
        best = (INF, ())
        for b in range(a + 1, Wn):
            if rest >> b & 1:
                c, pl = f(rest & ~(1 << b), single_used)
                if M[a, b] + c < best[0]:
                    best = (M[a, b] + c, pl + ((a, b),))
        if not single_used:
            c, pl = f(rest, True)
            if s[a] + c < best[0]:
                best = (s[a] + c, pl + ((a, None),))
        return best

    c, pl = f((1 << Wn) - 1, False)
    f.cache_clear()
    return c, list(pl)


def _build_plan():
    psi, ti_idx, W = _compute_psi()
    dpval = np.where(np.arange(P) < P // 2, np.arange(P), np.arange(P) - P)
    active = (psi != 0).any(axis=0)  # [To, W, P]

    cnt = active.reshape(NLAT, -1).sum(axis=1)
    order = list(np.argsort(-cnt, kind="stable"))
    n = len(order)
    INF = 10**12
    cost = np.full((n + 1, n + 1), INF, dtype=np.int64)
    pairings = {}
    for i in range(n):
        u = np.zeros((W, P), dtype=bool)
        for j in range(i + 1, min(i + 9, n + 1)):
            u = u | active[order[j - 1]]
            c, pl = _best_matching(u)
            cost[i][j] = c
            pairings[(i, j)] = pl
    dp = np.full((n + 1, NSLOT + 1), INF, dtype=np.int64)
    par = np.zeros((n + 1, NSLOT + 1), dtype=np.int64)
    dp[0][0] = 0
    for j in range(1, NSLOT + 1):
        for i in range(1, n + 1):
            for i0 in range(max(0, i - 8), i):
                v = dp[i0][j - 1] + cost[i0][i]
                if v < dp[i][j]:
                    dp[i][j] = v
                    par[i][j] = i0
    bounds = []
    i = n
    for j in range(NSLOT, 0, -1):
        i0 = par[i][j]
        bounds.append((i0, i))
        i = i0
    bounds = bounds[::-1]

    row_of = -np.ones((NCORE, NSLOT), dtype=np.int64)
    slot_pairs, templates, halos = [], [], []
    for s, (i0, i1) in enumerate(bounds):
        rows = [order[t] for t in range(i0, i1)]
        for ci, t in enumerate(rows):
            row_of[ci, s] = t
        pairs = pairings[(i0, i1)]
        assert len(pairs) == NJ
        slot_pairs.append(pairs)
        u = active[rows].any(axis=0)  # [W, P]
        tap_list = []
        for j, (wa, wb) in enumerate(pairs):
            ws = [w for w in (wa, wb) if w is not None]
            act_j = u[ws].any(axis=0)  # [P]
            pp = np.nonzero(act_j)[0]
            for dp_ in sorted(dpval[pp].tolist()):
                tap_list.append((j, dp_))
        templates.append(tap_list)
        halos.append(max((abs(d) for _, d in tap_list), default=0))

    qpads = [P + 2 * h for h in halos]
    offs = np.cumsum([0] + [NJ * B * qp for qp in qpads]).tolist()

    # --- tap pairing: fuse (j,dp) with (j,dp+1) into one M=128 matmul ---
    # per slot: list of ('P', j, dp) pairs first (any pair initializes the
    # full [128, NPAIR] psum region via start=True), then ('S', j, dp).
    tgmap = {}
    tg = 0
    for s in range(NSLOT):
        for (j, dp_) in templates[s]:
            tgmap[(s, j, dp_)] = tg
            tg += 1
    from collections import defaultdict
    slot_descs = []
    wp_col = 0
    for s in range(NSLOT):
        byj = defaultdict(list)
        for j, dp_ in templates[s]:
            byj[j].append(dp_)
        prs, sgl = [], []
        for j in sorted(byj):
            dps = sorted(byj[j])
            i = 0
            while i < len(dps):
                if i + 1 < len(dps) and dps[i + 1] == dps[i] + 1:
                    prs.append((j, dps[i]))
                    i += 2
                else:
                    sgl.append((j, dps[i]))
                    i += 1
        assert prs, f"slot {s} has no paired tap"
        descs = []
        for j, dp_ in prs:
            descs.append(("P", j, dp_, wp_col))
            wp_col += 2 * O
        for j, dp_ in sgl:
            descs.append(("S", j, dp_, wp_col))
            wp_col += O
        slot_descs.append(descs)

    # wp chunk boundaries (in cols), aligned to matmul blocks
    flat = [d for ds in slot_descs for d in ds]
    chunk_bounds = [0]
    target = WP_FIRST
    for kind, _, _, col in flat:
        ncols = 2 * O if kind == "P" else O
        if col + ncols - chunk_bounds[-1] > target:
            chunk_bounds.append(col)
            target = WP_CHUNK
    chunk_bounds.append(wp_col)

    return dict(psi=psi, ti_idx=ti_idx, W=W, row_of=row_of, templates=templates,
                slot_pairs=slot_pairs, halos=halos, qpads=qpads, offs=offs,
                xg_cols=int(offs[-1]), tgmap=tgmap, slot_descs=slot_descs,
                wp_cols=int(wp_col), chunk_bounds=chunk_bounds,
                t_total=int(sum(len(t) for t in templates)))


_PLAN = None
_NC = None


def _get_plan():
    global _PLAN
    if _PLAN is None:
        _PLAN = _build_plan()
    return _PLAN


def _build_nc(plan):
    import concourse.bacc as bacc
    import concourse.mybir as mybir
    import concourse.tile as tile

    f32 = mybir.dt.float32
    bf16 = mybir.dt.bfloat16

    halos = plan["halos"]
    qpads = plan["qpads"]
    offs = plan["offs"]
    XG_COLS = plan["xg_cols"]
    WP_COLS = plan["wp_cols"]
    slot_descs = plan["slot_descs"]
    chunk_bounds = plan["chunk_bounds"]

    nc = bacc.Bacc("TRN2", target_bir_lowering=False, debug=False,
                   num_devices=NCORE)
    xg_d = nc.declare_dram_parameter("xg", [128, XG_COLS], bf16, isOutput=False)
    wp_d = nc.declare_dram_parameter("wp", [128, WP_COLS], bf16, isOutput=False)
    # both tap halves are shipped out; the host does the 1-longitude
    # shifted add (cross-partition adds are rejected by the BIR verifier)
    out_d = nc.declare_dram_parameter("out", [128, NSLOT * NPAIR], f32,
                                      isOutput=True)

    with tile.TileContext(nc) as tc:
        with (
            tc.tile_pool(name="xg", bufs=1) as xgp,
            tc.tile_pool(name="wp", bufs=3) as wpp,
            tc.tile_pool(name="ps", bufs=4, space="PSUM") as psp,
            tc.tile_pool(name="outp", bufs=2) as outp,
        ):
            # xg tiles on the gpsimd (SWDGE) queue so inputs load in
            # parallel with the weight chunks (sync/HWDGE). Slot 0 is split
            # per pair-slot j so the very first matmul only waits on a sliver.
            xg_ts = []
            for s in range(NSLOT):
                qp = qpads[s]
                if s == 0:
                    pieces = []
                    for j in range(NJ):
                        pj = xgp.tile([128, B * qp], bf16, tag=f"xg0_{j}")
                        nc.gpsimd.dma_start(
                            pj[:], xg_d[:, offs[s] + j * B * qp:
                                        offs[s] + (j + 1) * B * qp])
                        pieces.append(pj)
                    xg_ts.append(pieces)
                else:
                    seg = xgp.tile([128, NJ * B * qp], bf16, tag=f"xg{s}")
                    nc.gpsimd.dma_start(seg[:], xg_d[:, offs[s]:offs[s + 1]])
                    xg_ts.append(seg)
            out_t = outp.tile([128, NSLOT * NPAIR], f32)

            cidx = 0
            wp_t = None
            for s in range(NSLOT):
                descs = slot_descs[s]
                qp = qpads[s]
                h = halos[s]
                acc = psp.tile([128, NPAIR], f32)
                for i, (kind, j, dp, col) in enumerate(descs):
                    ncols = 2 * O if kind == "P" else O
                    if col == chunk_bounds[cidx]:
                        ccols = chunk_bounds[cidx + 1] - chunk_bounds[cidx]
                        wp_t = wpp.tile([128, WP_CHUNK], bf16, tag="wp")
                        nc.sync.dma_start(
                            wp_t[:, :ccols],
                            wp_d[:, chunk_bounds[cidx]:chunk_bounds[cidx + 1]])
                        cidx += 1
                    coff = col - chunk_bounds[cidx - 1]
                    lhsT = wp_t[:, coff:coff + ncols]
                    if s == 0:
                        xv = xg_ts[0][j]
                        base = B * (h + dp)
                    else:
                        xv = xg_ts[s]
                        base = j * B * qp + B * (h + dp)
                    if kind == "P":
                        rhs = xv[:, base: base + NPAIR]
                        out_ap = acc[:, :]
                    else:
                        rhs = xv[:, base: base + B * P]
                        out_ap = acc[0:O, 0:B * P]
                    nc.tensor.matmul(out_ap, lhsT, rhs,
                                     start=(i == 0), stop=(i == len(descs) - 1))
                nc.vector.tensor_copy(
                    out_t[:, s * NPAIR:(s + 1) * NPAIR], acc[:, :])
                # output DMAs ride the scalar-engine queue so they never
                # delay weight chunks
                nc.scalar.dma_start(
                    out_d[:, s * NPAIR:(s + 1) * NPAIR],
                    out_t[:, s * NPAIR:(s + 1) * NPAIR])

    nc.compile()
    return nc


def _get_nc():
    global _NC
    if _NC is None:
        _NC = _build_nc(_get_plan())
    return _NC


def _build_core_inputs(plan, x, weight):
    import ml_dtypes

    psi = plan["psi"]
    ti_idx = plan["ti_idx"]
    row_of = plan["row_of"]
    templates = plan["templates"]
    slot_pairs = plan["slot_pairs"]
    halos = plan["halos"]
    qpads = plan["qpads"]
    offs = plan["offs"]
    XG_COLS = plan["xg_cols"]
    T = plan["t_total"]
    tgmap = plan["tgmap"]
    slot_descs = plan["slot_descs"]
    WP_COLS = plan["wp_cols"]

    coef = np.zeros((NCORE, T, 2, K), dtype=np.float32)
    tg = 0
    for s in range(NSLOT):
        pairs = slot_pairs[s]
        for (j, dp) in templates[s]:
            p = dp % P
            members = pairs[j]
            for core in range(NCORE):
                to = row_of[core, s]
                if to < 0:
                    continue
                for m in range(2):
                    w_ = members[m] if m < len(members) else None
                    if w_ is not None:
                        coef[core, tg, m] = psi[:, to, w_, p]
            tg += 1
    wk = np.ascontiguousarray(weight.transpose(2, 1, 0)).reshape(K, C, O)
    # [NCORE, T, 2, C, O] -> per tap a [128=(m,c), 64=o] stationary block
    wp_all = np.einsum("ntmk,kco->ntmco", coef, wk, optimize=True)

    # assemble wp in matmul-stream order (pair blocks: tap dp then dp+1)
    tg_order = []
    for s in range(NSLOT):
        for kind, j, dp, _ in slot_descs[s]:
            tg_order.append(tgmap[(s, j, dp)])
            if kind == "P":
                tg_order.append(tgmap[(s, j, dp + 1)])
    tg_order = np.array(tg_order, dtype=np.int64)
    wps = []
    for n in range(NCORE):
        blocks = wp_all[n][tg_order].reshape(len(tg_order), 128, O)
        wp = np.ascontiguousarray(blocks.transpose(1, 0, 2).reshape(128, -1))
        assert wp.shape[1] == WP_COLS
        wps.append(wp.astype(ml_dtypes.bfloat16))

    xgs = []
    for core in range(NCORE):
        xg = np.zeros((128, XG_COLS), dtype=np.float32)
        for s in range(NSLOT):
            to = row_of[core, s]
            if to < 0:
                continue
            qp = qpads[s]
            h = halos[s]
            qq = (np.arange(qp) - h) % P
            for j, members in enumerate(slot_pairs[s]):
                for m in range(2):
                    w_ = members[m] if m < len(members) else None
                    if w_ is None:
                        continue
                    ti = ti_idx[to, w_]
                    blk = x[:, :, ti, :][:, :, qq]  # [b, c, qp]
                    # column layout (qq, b) so a dp-shifted rhs is contiguous
                    xg[m * 64:(m + 1) * 64,
                       offs[s] + j * B * qp: offs[s] + (j + 1) * B * qp] = (
                        blk.transpose(1, 2, 0).reshape(C, qp * B))
        xgs.append(xg.astype(ml_dtypes.bfloat16))
    return xgs, wps


def kernel(x, weight):
    from concourse.bass_utils import run_bass_kernel_spmd

    x = np.ascontiguousarray(np.asarray(x, dtype=np.float32))
    weight = np.ascontiguousarray(np.asarray(weight, dtype=np.float32))
    plan = _get_plan()
    nc = _get_nc()
    xgs, wps = _build_core_inputs(plan, x, weight)
    in_maps = [{"xg": xgs[i], "wp": wps[i]} for i in range(NCORE)]
    res = run_bass_kernel_spmd(nc, in_maps, list(range(NCORE)))

    out = np.zeros((B, O, NLAT, P), dtype=np.float32)
    row_of = plan["row_of"]
    for core in range(NCORE):
        oc = np.asarray(res.results[core]["out"]).reshape(128, NSLOT, NPAIR)
        merged = oc[0:O, :, 0:B * P] + oc[O:128, :, B:B * P + B]
        mg = merged.reshape(O, NSLOT, P, B)
        for s in range(NSLOT):
            to = row_of[core, s]
            if to >= 0:
                out[:, :, to, :] = mg[:, s, :, :].transpose(2, 0, 1)
    return out


def _numpy_sim(x, weight):
    """Host replica of the device program (for validation)."""
    plan = _get_plan()
    xgs, wps = _build_core_inputs(plan, x, weight)
    slot_descs = plan["slot_descs"]
    halos = plan["halos"]
    qpads = plan["qpads"]
    offs = plan["offs"]
    out = np.zeros((B, O, NLAT, P), dtype=np.float32)
    row_of = plan["row_of"]
    for core in range(NCORE):
        xg = xgs[core].astype(np.float32)
        wp = wps[core].astype(np.float32)
        oc = np.zeros((O, NSLOT, P, B), dtype=np.float32)
        for s in range(NSLOT):
            qp = qpads[s]
            h = halos[s]
            acc = np.zeros((128, NPAIR), dtype=np.float32)
            for (kind, j, dp, col) in slot_descs[s]:
                base = offs[s] + j * B * qp + B * (h + dp)
                if kind == "P":
                    rhs = xg[:, base: base + NPAIR]
                    acc += wp[:, col:col + 2 * O].T @ rhs
                else:
                    rhs = xg[:, base: base + B * P]
                    acc[0:O, 0:B * P] += wp[:, col:col + O].T @ rhs
            merged = acc[0:O, 0:B * P] + acc[O:2 * O, B:B * P + B]
            oc[:, s] = merged.reshape(O, P, B)
        for s in range(NSLOT):
            to = row_of[core, s]
            if to >= 0:
                out[:, :, to, :] = oc[:, s, :, :].transpose(2, 0, 1)
    return out


if __name__ == "__main__":
    plan = _get_plan()
    npair = sum(1 for ds in plan["slot_descs"] for d in ds if d[0] == "P")
    nsgl = sum(1 for ds in plan["slot_descs"] for d in ds if d[0] == "S")
    cyc = npair * NPAIR + nsgl * B * P
    print("t_total:", plan["t_total"], "pairs:", npair, "singles:", nsgl,
          "matmul cycles:", cyc, "->", cyc * 0.41667 / 1e3, "us")
    print("xg MB:", plan["xg_cols"] * 128 * 2 / 1e6,
          "wp MB:", plan["wp_cols"] * 128 * 2 / 1e6)
    d = np.load("/tmp/ref_io.npz")
    got = _numpy_sim(d["x"], d["weight"])
    exp = d["expected"]
    rel = np.linalg.norm((got - exp).ravel()) / np.linalg.norm(exp.ravel())
    print("numpy-sim rel err:", rel)
